# revision 9
# baseline (speedup 1.0000x reference)
"""Trainium2 Bass kernel for nn_ExtractModel (retrieval_knn).

Strategy: vocab axis NT=8000 sharded across 8 cores (~1000 each, sorted by
vocab_length so banded-DP cells shrink to the vlen-prefix that can use them).
Per core the device program:
  Phase A: word/unit feature-sum embeddings via one-hot matmuls on PE,
           norms + cosine-distance tables, shifted per-candidate-offset
           tables cosE[u, w, pos].
  Phase B: banded soft-edit-distance DP over [512 src positions x NT_shard],
           substitution costs gathered via one-hot matmul on PE (PSUM f32),
           converted to fp16 on ACT, DP min/add on DVE (fp16, 2x mode).
  Output:  per-(ls,vlen-group) min-reduce -> best_value partial [512, 7].
Host: min across cores, then exact f32 replication of the reference epilogue
(score/argmax/threshold), which only consumes best_value.
"""

import os
import numpy as np

# Problem constants (hardcoded; kernel.py must be self-contained).
MIN_WL, MAX_WL = 4, 10
MSL, MTL = 10, 10
THRESHOLD = 0.05
B, L, NT, U, G, NF, D = 8, 64, 8000, 64, 6, 512, 256
LEN_E = MAX_WL + 1 - MIN_WL  # 7
BIG = np.float32(99.9)
N_CORES = 8
NS = B * L          # 512
NBLK = NS // 128    # 4
MMCHUNK = 512       # matmul moving-operand free-dim max (fp32-safe)

_LAST = {"exec_time_ns": None, "results": None}


# --------------------------------------------------------------------------
# Host-side integer prep
# --------------------------------------------------------------------------

def _host_prep(feat_matrix, lengths, unit_feat_matrix, indexed_segments,
               vocab_length):
    feat = np.asarray(feat_matrix).astype(np.int64)
    lens = np.asarray(lengths).astype(np.int64)
    ufeat = np.asarray(unit_feat_matrix).astype(np.int64)
    seg = np.asarray(indexed_segments).astype(np.int64)
    vlen = np.asarray(vocab_length).astype(np.int64)

    # onehotF[f, bl]: multiplicity of feature f at source position bl=(b,l),
    # zeroed for padded positions (l >= lengths[b]) -- folds the src_pad mask.
    ohf = np.zeros((NF, NS), dtype=np.float32)
    for b in range(B):
        for l in range(L):
            if l < lens[b]:
                for g in range(G):
                    ohf[feat[b, l, g], b * L + l] += 1.0

    ohu = np.zeros((NF, U), dtype=np.float32)
    for u in range(U):
        for g in range(G):
            ohu[ufeat[u, g], u] += 1.0

    # Sort vocab by vlen desc; shard so every core has identical group sizes
    # C_v (shortfall padded with duplicate entries -- min() is idempotent).
    counts = {v: int((vlen == v).sum()) for v in range(MIN_WL, MAX_WL + 1)}
    cv = {v: (counts[v] + N_CORES - 1) // N_CORES for v in counts}
    ntc = sum(cv.values())
    # group offsets in descending-v order
    offs = {}
    off = 0
    for v in range(MAX_WL, MIN_WL - 1, -1):
        offs[v] = off
        off += cv[v]
    # W(lt): number of shard columns whose vlen >= lt (prefix width)
    W = {}
    for lt in range(1, MTL + 1):
        W[lt] = sum(cv[v] for v in range(max(lt, MIN_WL), MAX_WL + 1))

    idx_by_v = {v: np.nonzero(vlen == v)[0] for v in counts}
    ohs_cores = []
    for c in range(N_CORES):
        cols = []
        for v in range(MAX_WL, MIN_WL - 1, -1):
            n_v, c_v = counts[v], cv[v]
            if c_v == 0:
                continue
            take = [(c * c_v + i) % n_v for i in range(c_v)]
            cols.append(idx_by_v[v][take])
        cols = np.concatenate(cols) if cols else np.zeros(0, np.int64)
        assert cols.shape[0] == ntc
        oh = np.zeros((MTL, U, ntc), dtype=np.float16)
        for lt in range(1, MTL + 1):
            w = W[lt]
            if w:
                oh[lt - 1, seg[cols[:w], lt - 1], np.arange(w)] = 1.0
        ohs_cores.append(oh)

    return ohf, ohu, ohs_cores, ntc, cv, offs, W


# --------------------------------------------------------------------------
# Device program
# --------------------------------------------------------------------------

def _build_program(ntc, cv, offs, W):
    from contextlib import ExitStack
    import concourse.bacc as bacc
    import concourse.bass as bass
    import concourse.mybir as mybir
    import concourse.tile as tile

    dt = mybir.dt
    Alu = mybir.AluOpType
    Act = mybir.ActivationFunctionType
    AxX = mybir.AxisListType.X

    nc = bacc.Bacc("TRN2", target_bir_lowering=False, debug=False)

    emb_d = nc.dram_tensor("emb", [NF, D], dt.float32, kind="ExternalInput")
    ohf_d = nc.dram_tensor("ohf", [NF, NS], dt.float32, kind="ExternalInput")
    ohu_d = nc.dram_tensor("ohu", [NF, U], dt.float32, kind="ExternalInput")
    ohs_d = nc.dram_tensor("ohs", [MTL, U, ntc], dt.float16,
                           kind="ExternalInput")
    id_d = nc.dram_tensor("id128", [128, 128], dt.float32,
                          kind="ExternalInput")
    bv_d = nc.dram_tensor("bv", [NBLK, 128, LEN_E], dt.float32,
                          kind="ExternalOutput")

    KT = NF // 128  # 4 k-tiles over features
    DT = D // 128   # 2 tiles over embedding dim

    with tile.TileContext(nc) as tc, ExitStack() as ctx:
        const = ctx.enter_context(tc.tile_pool(name="const", bufs=1))
        work = ctx.enter_context(tc.tile_pool(name="work", bufs=1))
        state = ctx.enter_context(tc.tile_pool(name="state", bufs=2))
        sh = ctx.enter_context(tc.tile_pool(name="sh", bufs=4))
        dpool = ctx.enter_context(tc.tile_pool(name="dp", bufs=3))
        psA_ctx = tc.tile_pool(name="psA", bufs=1, space="PSUM")
        psA = psA_ctx.__enter__()

        # ---- load inputs --------------------------------------------------
        emb_sb = const.tile([128, KT, D], dt.float32)
        nc.sync.dma_start(emb_sb[:], emb_d[:].rearrange("(k p) d -> p k d",
                                                        p=128))
        ohf_sb = const.tile([128, KT, NS], dt.float32)
        nc.sync.dma_start(ohf_sb[:], ohf_d[:].rearrange("(k p) n -> p k n",
                                                        p=128))
        ohu_sb = const.tile([128, KT, U], dt.float32)
        nc.sync.dma_start(ohu_sb[:], ohu_d[:].rearrange("(k p) n -> p k n",
                                                        p=128))
        ohs_sb = const.tile([U, MTL, ntc], dt.float16)
        nc.sync.dma_start(ohs_sb[:], ohs_d[:].rearrange("t u n -> u t n"))
        id_sb = const.tile([128, 128], dt.float32)
        nc.sync.dma_start(id_sb[:], id_d[:])

        # ---- Phase A: cosine tables --------------------------------------
        # wordT[d, bl] = sum_f emb[f, d] * ohf[f, bl]
        wT_sb = work.tile([128, DT, NS], dt.float32, tag="wT")
        wsq_sb = work.tile([128, DT, NS], dt.float32, tag="wsq")
        for m in range(DT):
            ps = psA.tile([128, NS], dt.float32, tag="psA")
            for k in range(KT):
                nc.tensor.matmul(ps[:], emb_sb[:, k, m * 128:(m + 1) * 128],
                                 ohf_sb[:, k, :],
                                 start=(k == 0), stop=(k == KT - 1))
            nc.scalar.copy(wT_sb[:, m, :], ps[:])
            nc.scalar.activation(wsq_sb[:, m, :], ps[:], Act.Square)

        # unit[u, d] = sum_f ohu[f, u] * emb[f, d]
        unit_ps = psA.tile([U, D], dt.float32, tag="psA")
        for k in range(KT):
            nc.tensor.matmul(unit_ps[:], ohu_sb[:, k, :], emb_sb[:, k, :],
                             start=(k == 0), stop=(k == KT - 1))
        unit_sb = work.tile([U, D], dt.float32, tag="unit")
        usq_sb = work.tile([U, D], dt.float32, tag="usq")
        n2u = work.tile([U, 1], dt.float32, tag="n2u")
        nc.scalar.copy(unit_sb[:], unit_ps[:])
        nc.scalar.activation(usq_sb[:], unit_ps[:], Act.Square,
                             accum_out=n2u[:])
        invny = work.tile([U, 1], dt.float32, tag="invny")
        nc.scalar.activation(invny[:], n2u[:], Act.Sqrt)
        nc.vector.tensor_scalar_add(invny[:], invny[:], 1e-8)
        nc.vector.reciprocal(invny[:], invny[:])

        # ||word||^2 per bl as a row vector via ones-matmul over wsq.
        ones_sb = const.tile([128, 1], dt.float32)
        nc.vector.memset(ones_sb[:], 1.0)
        n2r_ps = psA.tile([1, NS], dt.float32, tag="n2r")
        for m in range(DT):
            nc.tensor.matmul(n2r_ps[:], ones_sb[:], wsq_sb[:, m, :],
                             start=(m == 0), stop=(m == DT - 1))
        invnx_row = work.tile([1, NS], dt.float32, tag="invnx")
        nc.scalar.activation(invnx_row[:], n2r_ps[:], Act.Sqrt)
        nc.vector.tensor_scalar_add(invnx_row[:], invnx_row[:], 1e-8)
        nc.vector.reciprocal(invnx_row[:], invnx_row[:])
        # broadcast invnx over U partitions via K=1 matmul
        ones_u = const.tile([1, U], dt.float32)
        nc.vector.memset(ones_u[:], 1.0)
        bc_ps = psA.tile([U, NS], dt.float32, tag="bc")
        nc.tensor.matmul(bc_ps[:], ones_u[:], invnx_row[:],
                         start=True, stop=True)

        # unitT[d, u] via PE transpose
        unitT_sb = work.tile([128, DT, U], dt.float32, tag="unitT")
        for m in range(DT):
            tp = psA.tile([128, U], dt.float32, tag="tp")
            nc.tensor.transpose(tp[:], unit_sb[:, m * 128:(m + 1) * 128],
                                id_sb[0:U, 0:U])
            nc.scalar.copy(unitT_sb[:, m, :], tp[:])

        # dotT[u, bl] then scale by invny (per-partition) and invnx (bcast)
        dot_ps = psA.tile([U, NS], dt.float32, tag="dot")
        for m in range(DT):
            nc.tensor.matmul(dot_ps[:], unitT_sb[:, m, :], wT_sb[:, m, :],
                             start=(m == 0), stop=(m == DT - 1))
        tmp_sb = work.tile([U, NS], dt.float32, tag="tmp")
        nc.vector.tensor_scalar_mul(tmp_sb[:], dot_ps[:], invny[:])
        cospre = work.tile([U, NS], dt.float32, tag="cospre")
        nc.vector.tensor_tensor(cospre[:], tmp_sb[:], bc_ps[:], op=Alu.mult)
        # cos = 0.5 - 0.5 * (dot * invnx * invny), fp16
        cosT = work.tile([U, NS], dt.float16, tag="cosT")
        nc.scalar.activation(cosT[:], cospre[:], Act.Copy, bias=0.5,
                             scale=-0.5)

        # cosE[u, w, bl]: source char at offset w from position bl, clamped
        cosE = const.tile([U, MSL, NS], dt.float16)
        cosT_v = cosT[:].rearrange("u (b l) -> u b l", b=B)
        cosE_v = cosE[:].rearrange("u w (b l) -> u w b l", b=B)
        for w in range(MSL):
            if w == 0:
                nc.sync.dma_start(cosE[:, 0, :], cosT[:])
            else:
                nc.sync.dma_start(cosE_v[:, w, :, 0:L - w],
                                  cosT_v[:, :, w:L])
                for j in range(L - w, L):
                    nc.sync.dma_start(cosE_v[:, w, :, j:j + 1],
                                      cosT_v[:, :, L - 1:L])

        # ---- Phase B: banded DP over vocab shard -------------------------
        psA_ctx.__exit__(None, None, None)  # free phase-A PSUM banks
        psum = ctx.enter_context(tc.tile_pool(name="psum", bufs=8,
                                              space="PSUM"))
        f16 = dt.float16

        def band(ls):
            return range(max(ls - 2, 1), min(ls + 2, MTL + 1))

        bv_cols = {}  # ls -> list of vlen group values extracted
        for ls in range(MIN_WL, MAX_WL + 1):
            bv_cols[ls] = [v for v in range(max(MIN_WL, ls - 2),
                                            min(MAX_WL, ls + 1) + 1)
                           if cv[v] > 0]

        for blk in range(NBLK):
            bvp = dpool.tile([128, LEN_E, 4], dt.float32, tag="bvp")
            nc.vector.memset(bvp[:], float(BIG))

            prev = {}
            for ls in range(1, MSL + 1):
                lhsT = cosE[:, ls - 1, blk * 128:(blk + 1) * 128]
                # gather substitution costs for this row's band
                diff = {}
                for lt in band(ls):
                    wlt = W[lt]
                    dtile = dpool.tile([128, W[max(ls - 2, 1)]], f16,
                                       tag=f"diff{lt - ls}")
                    for c0 in range(0, wlt, MMCHUNK):
                        cw = min(MMCHUNK, wlt - c0)
                        ps = psum.tile([128, MMCHUNK], dt.float32, tag="mm")
                        nc.tensor.matmul(ps[:, 0:cw], lhsT,
                                         ohs_sb[:, lt - 1, c0:c0 + cw],
                                         start=True, stop=True)
                        nc.scalar.copy(dtile[:, c0:c0 + cw], ps[:, 0:cw])
                    diff[lt] = dtile

                cur = {}
                for lt in band(ls):
                    wlt = W[lt]
                    ct = state.tile([128, W[max(ls - 2, 1)]], f16,
                                    tag=f"st{lt - ls}")
                    d_ = diff[lt][:, 0:wlt]
                    o_ = ct[:, 0:wlt]
                    lo = max(ls - 2, 1)
                    if ls == 1:
                        if lt == 1:  # min(2, d)
                            nc.vector.tensor_scalar_min(o_, d_, 2.0)
                        else:        # lt=2: min(3, cur1+1, 1+d)
                            s_ = sh.tile([128, wlt], f16, tag="s")
                            nc.vector.tensor_scalar(s_[:, 0:wlt], d_, 1.0,
                                                    3.0, Alu.add, Alu.min)
                            nc.vector.scalar_tensor_tensor(
                                o_, cur[1][:, 0:wlt], 1.0, s_[:, 0:wlt],
                                Alu.add, Alu.min)
                    elif lt == 1:
                        # min(prev1+1, ls+1, (ls-1)+d)
                        s_ = sh.tile([128, wlt], f16, tag="s")
                        nc.vector.tensor_scalar(s_[:, 0:wlt], d_,
                                                float(ls - 1), float(ls + 1),
                                                Alu.add, Alu.min)
                        nc.vector.scalar_tensor_tensor(
                            o_, prev[1][:, 0:wlt], 1.0, s_[:, 0:wlt],
                            Alu.add, Alu.min)
                    elif lt == lo and lt > 1:
                        # bottom: min(prev[lt]+1, prev[lt-1]+d)
                        s_ = sh.tile([128, wlt], f16, tag="s")
                        nc.vector.tensor_tensor(s_[:, 0:wlt],
                                                prev[lt - 1][:, 0:wlt], d_,
                                                op=Alu.add)
                        nc.vector.scalar_tensor_tensor(
                            o_, prev[lt][:, 0:wlt], 1.0, s_[:, 0:wlt],
                            Alu.add, Alu.min)
                    elif lt == ls + 1 or lt not in prev:
                        # top: min(cur[lt-1]+1, prev[lt-1]+d)
                        s_ = sh.tile([128, wlt], f16, tag="s")
                        nc.vector.tensor_tensor(s_[:, 0:wlt],
                                                prev[lt - 1][:, 0:wlt], d_,
                                                op=Alu.add)
                        nc.vector.scalar_tensor_tensor(
                            o_, cur[lt - 1][:, 0:wlt], 1.0, s_[:, 0:wlt],
                            Alu.add, Alu.min)
                    else:
                        # full: min(min(prev[lt], cur[lt-1]) + 1,
                        #           prev[lt-1] + d)
                        s_ = sh.tile([128, wlt], f16, tag="s")
                        h_ = sh.tile([128, wlt], f16, tag="h")
                        nc.vector.tensor_tensor(s_[:, 0:wlt],
                                                prev[lt - 1][:, 0:wlt], d_,
                                                op=Alu.add)
                        nc.vector.scalar_tensor_tensor(
                            h_[:, 0:wlt], prev[lt][:, 0:wlt], 1.0,
                            s_[:, 0:wlt], Alu.add, Alu.min)
                        nc.vector.scalar_tensor_tensor(
                            o_, cur[lt - 1][:, 0:wlt], 1.0, h_[:, 0:wlt],
                            Alu.add, Alu.min)
                    cur[lt] = ct

                if ls >= MIN_WL:
                    for j, v in enumerate(bv_cols[ls]):
                        nc.vector.tensor_reduce(
                            bvp[:, ls - MIN_WL, j:j + 1],
                            cur[v][:, offs[v]:offs[v] + cv[v]],
                            AxX, Alu.min)
                prev = cur

            bv_sb = dpool.tile([128, LEN_E], dt.float32, tag="bvout")
            nc.vector.tensor_reduce(bv_sb[:], bvp[:], AxX, Alu.min)
            nc.sync.dma_start(bv_d[blk], bv_sb[:])

    nc.compile()
    return nc


# --------------------------------------------------------------------------
# Entry point
# --------------------------------------------------------------------------

def _run_spmd(nc, in_maps, n_cores, time_iters=0):
    """Execute the Bass module on n_cores via PJRT (axon), mirroring
    bass2jax.run_bass_via_pjrt but retaining the jitted callable so we can
    measure steady-state device execution time with repeated runs."""
    import jax
    import jax.numpy as jnp
    import concourse.mybir as mybir
    from jax.sharding import Mesh, PartitionSpec
    from jax.experimental.shard_map import shard_map
    from concourse.bass2jax import (_bass_exec_p, install_neuronx_cc_hook,
                                    partition_id_tensor)

    install_neuronx_cc_hook()
    partition_name = (nc.partition_id_tensor.name
                      if nc.partition_id_tensor else None)

    in_names, out_names, out_avals, zero_outs = [], [], [], []
    for alloc in nc.m.functions[0].allocations:
        if not isinstance(alloc, mybir.MemoryLocationSet):
            continue
        name = alloc.memorylocations[0].name
        if alloc.kind == "ExternalInput":
            if name != partition_name:
                in_names.append(name)
        elif alloc.kind == "ExternalOutput":
            shape = tuple(alloc.tensor_shape)
            dtype = mybir.dt.np(alloc.dtype)
            out_names.append(name)
            out_avals.append(jax.core.ShapedArray(shape, dtype))
            zero_outs.append(np.zeros(shape, dtype))
    n_params = len(in_names)
    all_in_names = list(in_names) + list(out_names)
    if partition_name is not None:
        all_in_names.append(partition_name)

    def _body(*args):
        operands = list(args)
        if partition_name is not None:
            operands.append(partition_id_tensor())
        return tuple(_bass_exec_p.bind(
            *operands,
            out_avals=tuple(out_avals),
            in_names=tuple(all_in_names),
            out_names=tuple(out_names),
            lowering_input_output_aliases=(),
            sim_require_finite=True,
            sim_require_nnan=True,
            nc=nc,
        ))

    devices = jax.devices()[:n_cores]
    mesh = Mesh(np.asarray(devices), ("core",))
    in_specs = (PartitionSpec("core"),) * (n_params + len(out_names))
    out_specs = (PartitionSpec("core"),) * len(out_names)
    sharded = jax.jit(shard_map(_body, mesh=mesh, in_specs=in_specs,
                                out_specs=out_specs, check_rep=False),
                      keep_unused=True)
    concat_in = [
        np.concatenate([np.asarray(in_maps[c][nm]) for c in range(n_cores)],
                       axis=0)
        for nm in in_names
    ]
    concat_zeros = [np.zeros((n_cores * z.shape[0], *z.shape[1:]), z.dtype)
                    for z in zero_outs]
    args = [jax.device_put(a) for a in concat_in + concat_zeros]
    out = sharded(*args)
    jax.block_until_ready(out)

    exec_ns = None
    if time_iters > 0:
        import time as _t
        # warmup
        for _ in range(3):
            jax.block_until_ready(sharded(*args))
        best = float("inf")
        for _ in range(5):
            t0 = _t.perf_counter()
            outs = [sharded(*args) for _ in range(time_iters)]
            jax.block_until_ready(outs)
            dt_ns = (_t.perf_counter() - t0) * 1e9 / time_iters
            best = min(best, dt_ns)
        exec_ns = best

    results = [
        {nm: np.asarray(out[i]).reshape(n_cores, *out_avals[i].shape)[c]
         for i, nm in enumerate(out_names)}
        for c in range(n_cores)
    ]
    return results, exec_ns


def kernel(emb, feat_matrix, lengths, unit_feat_matrix, indexed_segments,
           vocab_length):
    emb = np.ascontiguousarray(np.asarray(emb), dtype=np.float32)
    ohf, ohu, ohs_cores, ntc, cv, offs, W = _host_prep(
        feat_matrix, lengths, unit_feat_matrix, indexed_segments,
        vocab_length)

    nc = _build_program(ntc, cv, offs, W)

    id128 = np.eye(128, dtype=np.float32)
    in_maps = [
        {"emb": emb, "ohf": ohf, "ohu": ohu, "ohs": ohs_cores[c],
         "id128": id128}
        for c in range(N_CORES)
    ]

    time_iters = int(os.environ.get("BASSK_TIME_ITERS", "0"))
    results, exec_ns = _run_spmd(nc, in_maps, N_CORES,
                                 time_iters=time_iters)
    _LAST["exec_time_ns"] = exec_ns
    _LAST["results"] = results

    bv = np.min(np.stack([r["bv"] for r in results]), axis=0)
    bv = bv.reshape(NS, LEN_E).astype(np.float32).reshape(B, L, LEN_E)

    # Exact f32 epilogue (replicates reference._forward tail).
    lens = np.asarray(lengths).astype(np.int64)
    pos = np.arange(L)
    len_cand = MIN_WL + np.arange(LEN_E)
    end_cand = pos[:, None] + len_cand[None, :] - 1
    viable = end_cand[None] < lens[:, None, None]
    score = len_cand.astype(np.float32) * (np.float32(1.0) - bv)
    score = np.where(viable, score, np.float32(0.0))
    matched = viable & (bv < np.float32(THRESHOLD))
    flat = score.reshape(B, L * LEN_E)
    best_scores = flat.max(axis=-1)
    best_inds = flat.argmax(axis=-1)
    best_starts = best_inds // LEN_E
    best_ends = best_inds % LEN_E + best_starts + MIN_WL - 1
    matched_any = matched.reshape(B, -1).any(axis=-1)
    return (best_scores.astype(np.float32),
            best_starts.astype(np.int64),
            best_ends.astype(np.int64),
            matched_any)
